# revision 6
# baseline (speedup 1.0000x reference)
"""BF15IntLinear on 8 TRN2 NeuronCores.

Math: the reference quantizes x to "BF15" (truncate |x| toward zero to 6
explicit mantissa bits), W to truncated-bf16 (7 explicit bits), then does
an integer shift-align matmul whose result matches an exact
fp32-accumulated matmul of the quantized values to ~1e-5 relative — far
below the final bf16-cast ulp.  Both quantized operands are exactly
representable in bf16, and "truncate fp32 toward zero to bf16" is
literally "take the high uint16 of the fp32 word".

Kernel (per core; the 512x1024x1024 problem is sharded 2 M-groups x 4
N-groups):
  - contiguous fp32 DMA loads of the x / W shards ([128,1024] tiles)
  - TensorE transposes read the hi-uint16 lane of the fp32 tiles via
    stride-2 uint16 access patterns (integer datapath: bit-exact), giving
    K-partition-major quantized bf16-bit tiles with zero ALU prework
  - the PSUM->SBUF copy of the x tiles is a fused bitwise-AND 0xFFFE
    (clears the 7th mantissa bit -> BF15); W copies are plain
  - 16 bf16 matmuls accumulate in PSUM fp32
  - bias add (DVE) + cast to bf16 on the way out
"""

import numpy as np
import ml_dtypes

import concourse.bass as bass
import concourse.bacc as bacc
import concourse.mybir as mybir
from concourse import tile
from concourse.bass_utils import run_bass_kernel_spmd

# Problem shape (hardcoded per contract): x [4,128,1024] f32,
# weight [1024,1024] f32, bias [1024] f32 -> out [4,128,1024] bf16.
M, K, N = 512, 1024, 1024
M_GROUPS, N_GROUPS = 2, 4
M_SH, N_SH = M // M_GROUPS, N // N_GROUPS  # 256, 256
KB = K // 128  # 8 k-blocks
RT = M_SH // 128  # row-tiles per operand shard (2)

_CACHE: dict = {}


def _build_nc():
    dt = mybir.dt
    nc = bacc.Bacc("TRN2", debug=False, target_bir_lowering=False)
    x_d = nc.dram_tensor("x", [M_SH, K], dt.float32, kind="ExternalInput")
    w_d = nc.dram_tensor("w", [N_SH, K], dt.float32, kind="ExternalInput")
    b_d = nc.dram_tensor("b", [1, N_SH], dt.float32, kind="ExternalInput")
    i_d = nc.dram_tensor("ident", [128, 128], dt.bfloat16, kind="ExternalInput")
    y_d = nc.dram_tensor("y", [M_SH, N_SH], dt.bfloat16, kind="ExternalOutput")

    with tile.TileContext(nc) as tc:
        with (
            tc.tile_pool(name="sb", bufs=1) as pool,
            tc.tile_pool(name="ps", bufs=3, space=bass.MemorySpace.PSUM) as psum,
            tc.tile_pool(name="acc", bufs=1, space=bass.MemorySpace.PSUM) as psacc,
        ):
            idt = pool.tile([128, 128], dt.bfloat16, tag="idt")
            nc.sync.dma_start(out=idt[:, :], in_=i_d[:, :])

            bias_stage = pool.tile([1, N_SH], dt.float32, tag="bias_stage")
            nc.sync.dma_start(out=bias_stage[0:1, :], in_=b_d[:, :])
            bias_all = pool.tile([128, N_SH], dt.float32, tag="bias_all")
            nc.gpsimd.partition_broadcast(bias_all[:, :], bias_stage[0:1, :])

            # load fp32 shards; hi-u16 lane view = truncated-bf16 bit pattern
            xhi, whi = [], []
            for t in range(RT):
                for (src, dst) in ((x_d, xhi), (w_d, whi)):
                    f = pool.tile([128, K], dt.float32, tag=f"f{src.name}{t}")
                    nc.sync.dma_start(out=f[:, :], in_=src[t * 128:(t + 1) * 128, :])
                    hi = f[:, :].bitcast(dt.bfloat16).rearrange(
                        "p (k two) -> p k two", two=2
                    )[:, :, 1]
                    dst.append(hi)

            # transpose hi-lanes to K-partition-major via TensorE (u16 path)
            xt, wt = [], []
            for kb in range(KB):
                xtk = pool.tile([128, RT, 128], dt.bfloat16, tag=f"xt{kb}")
                wtk = pool.tile([128, RT, 128], dt.bfloat16, tag=f"wt{kb}")
                ptx = psum.tile([128, RT, 128], dt.bfloat16, tag="ptx")
                ptw = psum.tile([128, RT, 128], dt.bfloat16, tag="ptw")
                for t in range(RT):
                    nc.tensor.transpose(
                        ptx[:, t, :], xhi[t][:, kb * 128:(kb + 1) * 128], idt[:, :]
                    )
                    nc.tensor.transpose(
                        ptw[:, t, :], whi[t][:, kb * 128:(kb + 1) * 128], idt[:, :]
                    )
                # x: fused copy + BF15 mask (clear mantissa bit 7)
                nc.vector.tensor_scalar(
                    out=xtk[:, :, :].bitcast(dt.uint16), in0=ptx[:, :, :].bitcast(dt.uint16),
                    scalar1=0xFFFE, scalar2=None,
                    op0=mybir.AluOpType.bitwise_and,
                )
                nc.scalar.copy(wtk[:, :, :], ptw[:, :, :])
                xt.append(xtk)
                wt.append(wtk)

            # matmul: acc[mb] = sum_kb xt[kb][:,mb,:].T @ wt[kb]
            for mb in range(RT):
                acc = psacc.tile([128, N_SH], dt.float32, tag=f"acc{mb}")
                for kb in range(KB):
                    nc.tensor.matmul(
                        acc[:, :],
                        xt[kb][:, mb, :],
                        wt[kb][:, :, :],
                        start=(kb == 0),
                        stop=(kb == KB - 1),
                    )
                ysb = pool.tile([128, N_SH], dt.bfloat16, tag=f"y{mb}")
                nc.vector.tensor_tensor(
                    out=ysb[:, :], in0=acc[:, :], in1=bias_all[:, :],
                    op=mybir.AluOpType.add,
                )
                nc.sync.dma_start(out=y_d[mb * 128:(mb + 1) * 128, :], in_=ysb[:, :])

    nc.compile()
    return nc


def get_nc():
    if "nc" not in _CACHE:
        _CACHE["nc"] = _build_nc()
    return _CACHE["nc"]


def make_in_maps(x: np.ndarray, weight: np.ndarray, bias: np.ndarray):
    x2d = np.ascontiguousarray(x.reshape(M, K), dtype=np.float32)
    w = np.ascontiguousarray(weight, dtype=np.float32)
    b = np.ascontiguousarray(bias, dtype=np.float32)
    ident = np.eye(128, dtype=ml_dtypes.bfloat16)
    in_maps = []
    for c in range(M_GROUPS * N_GROUPS):
        mi, ni = divmod(c, N_GROUPS)
        in_maps.append({
            "x": np.ascontiguousarray(x2d[mi * M_SH:(mi + 1) * M_SH]),
            "w": np.ascontiguousarray(w[ni * N_SH:(ni + 1) * N_SH]),
            "b": np.ascontiguousarray(b[ni * N_SH:(ni + 1) * N_SH]).reshape(1, N_SH),
            "ident": ident,
        })
    return in_maps


def assemble(results) -> np.ndarray:
    y2d = np.empty((M, N), dtype=ml_dtypes.bfloat16)
    for c in range(M_GROUPS * N_GROUPS):
        mi, ni = divmod(c, N_GROUPS)
        y2d[mi * M_SH:(mi + 1) * M_SH, ni * N_SH:(ni + 1) * N_SH] = results[c]["y"]
    return y2d.reshape(4, 128, N)


def kernel(x: np.ndarray, weight: np.ndarray, bias: np.ndarray) -> np.ndarray:
    nc = get_nc()
    in_maps = make_in_maps(x, weight, bias)
    res = run_bass_kernel_spmd(nc, in_maps, core_ids=list(range(8)))
    return assemble(res.results)


# revision 7
# speedup vs baseline: 1.1321x; 1.1321x over previous
"""BF15IntLinear on 8 TRN2 NeuronCores.

Math: the reference quantizes x to "BF15" (truncate |x| toward zero to 6
explicit mantissa bits), W to truncated-bf16 (7 explicit bits), then does
an integer shift-align matmul whose result matches an exact
fp32-accumulated matmul of the quantized values to ~1e-5 relative — far
below the final bf16-cast ulp.  Both quantized operands are exactly
representable in bf16, and "truncate fp32 toward zero to bf16" is
literally "take the high uint16 of the fp32 word".

Kernel (per core; the 512x1024x1024 problem is sharded 2 M-groups x 4
N-groups):
  - two merged contiguous fp32 DMA loads (x shard, W shard), split across
    the two HWDGE trigger queues (sync + scalar)
  - TensorE transposes read the hi-uint16 lane of the fp32 tiles via
    stride-2 bf16 access patterns, giving K-partition-major quantized
    tiles with zero ALU prework; dummy transposes of the identity keep
    the PE busy during the DMA phase so the HAM clock gate is warm (2.4
    GHz) when the real matmuls run
  - the PSUM->SBUF copy of the x tiles is a fused bitwise-AND 0xFFFE
    (clears the 7th mantissa bit -> BF15); W copies are plain; all on DVE
  - 16 bf16 matmuls accumulate in PSUM fp32
  - bias (host-replicated to 128 partitions) add + cast to bf16 (DVE)
"""

import numpy as np
import ml_dtypes

import concourse.bass as bass
import concourse.bacc as bacc
import concourse.mybir as mybir
from concourse import tile
from concourse.bass_utils import run_bass_kernel_spmd

# Problem shape (hardcoded per contract): x [4,128,1024] f32,
# weight [1024,1024] f32, bias [1024] f32 -> out [4,128,1024] bf16.
M, K, N = 512, 1024, 1024
M_GROUPS, N_GROUPS = 2, 4
M_SH, N_SH = M // M_GROUPS, N // N_GROUPS  # 256, 256
KB = K // 128  # 8 k-blocks
RT = M_SH // 128  # row-tiles per operand shard (2)
N_WARM = 36  # dummy PE transposes to hold the HAM clock gate open

_CACHE: dict = {}


def _build_nc():
    dt = mybir.dt
    nc = bacc.Bacc("TRN2", debug=False, target_bir_lowering=False)
    x_d = nc.dram_tensor("x", [M_SH, K], dt.float32, kind="ExternalInput")
    w_d = nc.dram_tensor("w", [N_SH, K], dt.float32, kind="ExternalInput")
    b_d = nc.dram_tensor("b", [128, N_SH], dt.float32, kind="ExternalInput")
    i_d = nc.dram_tensor("ident", [128, 128], dt.bfloat16, kind="ExternalInput")
    y_d = nc.dram_tensor("y", [M_SH, N_SH], dt.bfloat16, kind="ExternalOutput")
    warm_d = nc.dram_tensor("warm", [1, 128], dt.bfloat16, kind="ExternalOutput")

    with tile.TileContext(nc) as tc:
        with (
            tc.tile_pool(name="sb", bufs=1) as pool,
            tc.tile_pool(name="ps", bufs=2, space=bass.MemorySpace.PSUM) as psum,
            tc.tile_pool(name="acc", bufs=1, space=bass.MemorySpace.PSUM) as psacc,
        ):
            # scalar-queue (HWDGE) loads: identity first so PE warmup can start
            idt = pool.tile([128, 128], dt.bfloat16, tag="idt")
            nc.scalar.dma_start(out=idt[:, :], in_=i_d[:, :])
            bias_all = pool.tile([128, N_SH], dt.float32, tag="bias_all")
            nc.scalar.dma_start(out=bias_all[:, :], in_=b_d[:, :])

            # sync-queue loads: one merged DMA per operand, [p, t, k] layout
            xf = pool.tile([128, RT, K], dt.float32, tag="xf")
            nc.sync.dma_start(
                out=xf[:, :, :], in_=x_d.ap().rearrange("(t p) k -> p t k", p=128)
            )
            wf = pool.tile([128, RT, K], dt.float32, tag="wf")
            nc.sync.dma_start(
                out=wf[:, :, :], in_=w_d.ap().rearrange("(t p) k -> p t k", p=128)
            )
            # hi-u16 lane views = truncated-bf16 bit patterns
            xhi = xf[:, :, :].bitcast(dt.bfloat16).rearrange(
                "p t (k two) -> p t k two", two=2
            )
            whi = wf[:, :, :].bitcast(dt.bfloat16).rearrange(
                "p t (k two) -> p t k two", two=2
            )

            # PE warmup: dummy transposes of the identity into a scratch bank.
            # Kept alive via a tiny DMA'd output so DCE can't drop the chain.
            wps = psum.tile([128, 128], dt.bfloat16, tag="wps")
            for _ in range(N_WARM):
                nc.tensor.transpose(wps[:, :], idt[:, :], idt[:, :])
            wsb = pool.tile([1, 128], dt.bfloat16, tag="wsb")
            nc.vector.tensor_copy(wsb[0:1, :], wps[0:1, :])
            nc.scalar.dma_start(out=warm_d[:, :], in_=wsb[0:1, :])

            # transpose hi-lanes to K-partition-major via TensorE
            xt, wt = [], []
            for kb in range(KB):
                xtk = pool.tile([128, RT, 128], dt.bfloat16, tag=f"xt{kb}")
                wtk = pool.tile([128, RT, 128], dt.bfloat16, tag=f"wt{kb}")
                ptx = psum.tile([128, RT, 128], dt.bfloat16, tag="ptx")
                ptw = psum.tile([128, RT, 128], dt.bfloat16, tag="ptw")
                for t in range(RT):
                    nc.tensor.transpose(
                        ptx[:, t, :], xhi[:, t, kb * 128:(kb + 1) * 128, 1], idt[:, :]
                    )
                    nc.tensor.transpose(
                        ptw[:, t, :], whi[:, t, kb * 128:(kb + 1) * 128, 1], idt[:, :]
                    )
                # x: fused copy + BF15 mask (clear mantissa bit 7)
                nc.vector.tensor_scalar(
                    out=xtk[:, :, :].bitcast(dt.uint16),
                    in0=ptx[:, :, :].bitcast(dt.uint16),
                    scalar1=0xFFFE, scalar2=None,
                    op0=mybir.AluOpType.bitwise_and,
                )
                nc.vector.tensor_copy(wtk[:, :, :], ptw[:, :, :])
                xt.append(xtk)
                wt.append(wtk)

            # matmul: acc[mb] = sum_kb xt[kb][:,mb,:].T @ wt[kb]
            ysb = pool.tile([128, RT, N_SH], dt.bfloat16, tag="ysb")
            for mb in range(RT):
                acc = psacc.tile([128, N_SH], dt.float32, tag=f"acc{mb}")
                for kb in range(KB):
                    nc.tensor.matmul(
                        acc[:, :],
                        xt[kb][:, mb, :],
                        wt[kb][:, :, :],
                        start=(kb == 0),
                        stop=(kb == KB - 1),
                    )
                nc.vector.tensor_tensor(
                    out=ysb[:, mb, :], in0=acc[:, :], in1=bias_all[:, :],
                    op=mybir.AluOpType.add,
                )
            nc.sync.dma_start(
                out=y_d.ap().rearrange("(mb p) n -> p mb n", p=128),
                in_=ysb[:, :, :],
            )

    nc.compile()
    return nc


def get_nc():
    if "nc" not in _CACHE:
        _CACHE["nc"] = _build_nc()
    return _CACHE["nc"]


def make_in_maps(x: np.ndarray, weight: np.ndarray, bias: np.ndarray):
    x2d = np.ascontiguousarray(x.reshape(M, K), dtype=np.float32)
    w = np.ascontiguousarray(weight, dtype=np.float32)
    b = np.ascontiguousarray(bias, dtype=np.float32)
    ident = np.eye(128, dtype=ml_dtypes.bfloat16)
    in_maps = []
    for c in range(M_GROUPS * N_GROUPS):
        mi, ni = divmod(c, N_GROUPS)
        bs = np.ascontiguousarray(
            np.broadcast_to(b[ni * N_SH:(ni + 1) * N_SH], (128, N_SH))
        )
        in_maps.append({
            "x": np.ascontiguousarray(x2d[mi * M_SH:(mi + 1) * M_SH]),
            "w": np.ascontiguousarray(w[ni * N_SH:(ni + 1) * N_SH]),
            "b": bs,
            "ident": ident,
        })
    return in_maps


def assemble(results) -> np.ndarray:
    y2d = np.empty((M, N), dtype=ml_dtypes.bfloat16)
    for c in range(M_GROUPS * N_GROUPS):
        mi, ni = divmod(c, N_GROUPS)
        y2d[mi * M_SH:(mi + 1) * M_SH, ni * N_SH:(ni + 1) * N_SH] = results[c]["y"]
    return y2d.reshape(4, 128, N)


def kernel(x: np.ndarray, weight: np.ndarray, bias: np.ndarray) -> np.ndarray:
    nc = get_nc()
    in_maps = make_in_maps(x, weight, bias)
    res = run_bass_kernel_spmd(nc, in_maps, core_ids=list(range(8)))
    return assemble(res.results)
